# revision 2
# baseline (speedup 1.0000x reference)
"""DMRGCN block kernel for 8 Trainium2 NeuronCores.

Problem: out = PReLU(TCN(PReLU(sum_r (x @ W_r) @ Anorm_r)) + x), where
Anorm = D^-1/2 (A+I) D^-1/2 per (b, r, t) graph of N=128 nodes.

Sharding: pure data parallel over B (64 samples -> 8 cores x 8).

Device pipeline per sample b (layouts: [partition, free]):
  XB  [65, (t n)]   <- DMA x[b] (+ ones row for the gcn bias)
  AB_r [128n, (t m)] <- DMA (A+I)[b, r]   (host adds I)
  DS/D [128n, (r t)]  rowsum -> d = rsqrt(rowsum)  (DVE reduce + Newton)
  mm1  (PE, fp32r): XC[n, (r o)] = x_bt^T @ WG per t  -> PSUM
  e0   (DVE): ZS[n, (r t o)] = XC * d[n]              (PSUM->SBUF)
  mm2  (PE, fp32):  PS2[m, o] = At_r^T @ ZS_rt per (r,t) -> PSUM
  e2   (DVE): Y2 = PS2 * d[m]                          (PSUM->SBUF)
  e3   (POOL): XCT[m, (t o)] = Y2[r=0] + Y2[r=1]
  tr   (PE): PS3[o, m] = XCT_t^T per t -> PSUM
  ph   (ACT): H[o, (t n)] = PReLU(PS3, a_tcn)  fp32r   (PSUM->SBUF)
  tcn  (PE, fp32r): PS4[o, (t n)] = I64 @ x  +  sum_k WT_k @ H(shift k)
  f2   (ACT): OUT = PReLU(PS4 + b_tcn, a_out)          (PSUM->SBUF)
  DMA OUT -> out[b]
"""

import numpy as np

B, R, T, N = 64, 2, 32, 128
C = 64
NCORES = 8
BL = B // NCORES  # samples per core
TN = T * N  # 4096
TCH = 4  # t-chunk size (4 t's = 512 psum cols)
NCH = T // TCH  # 8 chunks

_BUILD_CACHE = {}


def _build(a_tcn: float, a_out: float):
    key = (a_tcn, a_out)
    if key in _BUILD_CACHE:
        return _BUILD_CACHE[key]

    from contextlib import ExitStack

    import concourse.bacc as bacc
    import concourse.mybir as mybir
    import concourse.tile as tile

    dt = mybir.dt
    F32 = dt.float32
    F32R = dt.float32r

    nc = bacc.Bacc("TRN2", target_bir_lowering=False, debug=False)

    x_d = nc.dram_tensor("x", [BL, C + 1, TN], F32R, kind="ExternalInput")
    a_d = nc.dram_tensor("A", [BL, R, T, N, N], F32, kind="ExternalInput")
    wg_d = nc.dram_tensor("WG", [C + 1, 2 * C], F32R, kind="ExternalInput")
    wt_d = nc.dram_tensor("WT", [C, 3 * C], F32R, kind="ExternalInput")
    i64_d = nc.dram_tensor("I64", [C, C], F32R, kind="ExternalInput")
    i128_d = nc.dram_tensor("I128", [N, N], F32, kind="ExternalInput")
    bt_d = nc.dram_tensor("BT", [C, 1], F32, kind="ExternalInput")
    out_d = nc.dram_tensor("out", [BL, C, TN], F32, kind="ExternalOutput")

    with tile.TileContext(nc) as tc, ExitStack() as ctx:
        consts = ctx.enter_context(tc.tile_pool(name="consts", bufs=1))
        xp = ctx.enter_context(tc.tile_pool(name="xp", bufs=2))
        ap = ctx.enter_context(tc.tile_pool(name="ap", bufs=3))
        dp = ctx.enter_context(tc.tile_pool(name="dp", bufs=2))
        zsp = ctx.enter_context(tc.tile_pool(name="zsp", bufs=2))
        y2p = ctx.enter_context(tc.tile_pool(name="y2p", bufs=3))
        xctp = ctx.enter_context(tc.tile_pool(name="xctp", bufs=2))
        hp = ctx.enter_context(tc.tile_pool(name="hp", bufs=2))
        outp = ctx.enter_context(tc.tile_pool(name="outp", bufs=3))
        ps1p = ctx.enter_context(tc.tile_pool(name="ps1", bufs=2, space="PSUM"))
        ps2p = ctx.enter_context(tc.tile_pool(name="ps2", bufs=2, space="PSUM"))
        ps3p = ctx.enter_context(tc.tile_pool(name="ps3", bufs=2, space="PSUM"))
        ps4p = ctx.enter_context(tc.tile_pool(name="ps4", bufs=2, space="PSUM"))

        WG = consts.tile([C + 1, 2 * C], F32R)
        nc.sync.dma_start(WG[:], wg_d.ap())
        WT = consts.tile([C, 3 * C], F32R)
        nc.sync.dma_start(WT[:], wt_d.ap())
        I64 = consts.tile([C, C], F32R)
        nc.sync.dma_start(I64[:], i64_d.ap())
        I128 = consts.tile([N, N], F32)
        nc.sync.dma_start(I128[:], i128_d.ap())
        BT = consts.tile([C, 1], F32)
        nc.sync.dma_start(BT[:], bt_d.ap())

        Prelu = mybir.ActivationFunctionType.Prelu
        Sqrt = mybir.ActivationFunctionType.Sqrt
        mult = mybir.AluOpType.mult
        add = mybir.AluOpType.add

        for b in range(BL):
            XB = xp.tile([C + 1, TN], F32R, tag="xb")
            nc.sync.dma_start(XB[:], x_d.ap()[b])

            ABs = []
            for r in range(R):
                AB = ap.tile([N, TN], F32, tag="ab")
                nc.sync.dma_start(
                    AB[:].rearrange("n (t m) -> n t m", m=N),
                    a_d.ap()[b, r].rearrange("t n m -> n t m"),
                )
                ABs.append(AB)

            # ---- degree vector d = rsqrt(rowsum(A+I)) : [128, (r t)] ----
            DS = dp.tile([N, R * T], F32, tag="ds")
            for r in range(R):
                nc.vector.reduce_sum(
                    DS[:, r * T : (r + 1) * T],
                    ABs[r][:].rearrange("n (t m) -> n t m", m=N),
                    axis=mybir.AxisListType.X,
                )
            RC = dp.tile([N, R * T], F32, tag="rc")
            nc.vector.reciprocal(RC[:], DS[:])
            D0 = dp.tile([N, R * T], F32, tag="d0")
            nc.scalar.activation(D0[:], RC[:], Sqrt)
            # one Newton step for rsqrt: d = d0*(1.5 - 0.5*s*d0^2)
            T1 = dp.tile([N, R * T], F32, tag="t1")
            nc.vector.tensor_mul(T1[:], D0[:], D0[:])
            nc.vector.tensor_mul(T1[:], T1[:], DS[:])
            nc.vector.tensor_scalar(T1[:], T1[:], -0.5, 1.5, op0=mult, op1=add)
            D = dp.tile([N, R * T], F32, tag="d")
            nc.vector.tensor_mul(D[:], T1[:], D0[:])

            ZS = zsp.tile([N, R * T * C], F32, tag="zs")
            XCT = xctp.tile([N, T * C], F32, tag="xct")
            H = hp.tile([C, TN], F32R, tag="h")

            for j in range(NCH):
                t0 = j * TCH
                # ---- mm1: XC[n, (r o)] per t ----
                PS1 = ps1p.tile([N, TCH * 2 * C], F32, tag="ps1")
                for tt in range(TCH):
                    t = t0 + tt
                    nc.tensor.matmul(
                        PS1[:, tt * 128 : (tt + 1) * 128],
                        XB[:, t * N : (t + 1) * N],
                        WG[:],
                        start=True,
                        stop=True,
                    )
                # ---- e0: ZS = XC * d[n] ----
                ps1_v = PS1[:].rearrange("n (t r o) -> n t r o", t=TCH, r=R)
                zs_v = (
                    ZS[:]
                    .rearrange("n (r t o) -> n r t o", r=R, o=C)[
                        :, :, t0 : t0 + TCH
                    ]
                    .rearrange("n r t o -> n t r o")
                )
                d_v = (
                    D[:]
                    .rearrange("n (r t) -> n r t", r=R)[:, :, t0 : t0 + TCH]
                    .rearrange("n r t -> n t r")
                    .unsqueeze(3)
                    .broadcast_to((N, TCH, R, C))
                )
                nc.vector.tensor_tensor(zs_v, ps1_v, d_v, op=mult)

                # ---- mm2 per (r, t): PS2[m, o] ----
                PS2 = ps2p.tile([N, TCH * 2 * C], F32, tag="ps2")
                for tt in range(TCH):
                    t = t0 + tt
                    for r in range(R):
                        nc.tensor.matmul(
                            PS2[:, (tt * 2 + r) * C : (tt * 2 + r + 1) * C],
                            ABs[r][:, t * N : (t + 1) * N],
                            ZS[:, (r * T + t) * C : (r * T + t + 1) * C],
                            start=True,
                            stop=True,
                        )
                # ---- e2: Y2 = PS2 * d[m] ----
                Y2 = y2p.tile([N, TCH * 2 * C], F32, tag="y2")
                ps2_v = PS2[:].rearrange("m (t r o) -> m t r o", t=TCH, r=R)
                y2_v = Y2[:].rearrange("m (t r o) -> m t r o", t=TCH, r=R)
                nc.vector.tensor_tensor(y2_v, ps2_v, d_v, op=mult)
                # ---- e3 (gpsimd): XCT[m, (t o)] = Y2[r0] + Y2[r1] ----
                y2r = Y2[:].rearrange("m (t r o) -> m r t o", t=TCH, r=R)
                nc.gpsimd.tensor_tensor(
                    XCT[:, t0 * C : (t0 + TCH) * C].rearrange(
                        "m (t o) -> m t o", o=C
                    ),
                    y2r[:, 0],
                    y2r[:, 1],
                    op=add,
                )

                # ---- transpose per t: PS3[o, m] ----
                PS3 = ps3p.tile([C, TCH * N], F32, tag="ps3")
                for tt in range(TCH):
                    t = t0 + tt
                    nc.tensor.transpose(
                        PS3[:, tt * N : (tt + 1) * N],
                        XCT[:, t * C : (t + 1) * C],
                        I128[:],
                    )
                # ---- prelu-h (ACT): H = PReLU(PS3, a_tcn) ----
                nc.scalar.activation(
                    H[:, t0 * N : (t0 + TCH) * N], PS3[:], Prelu, alpha=a_tcn
                )

            # ---- tcn + residual per chunk ----
            for j in range(NCH):
                t0 = j * TCH
                t1 = t0 + TCH - 1  # inclusive
                PS4 = ps4p.tile([C, TCH * N], F32, tag="ps4")
                nc.tensor.matmul(
                    PS4[:],
                    I64[:],
                    XB[:C, t0 * N : (t1 + 1) * N],
                    start=True,
                    stop=False,
                )
                for k in range(3):
                    # valid out t range for tap k: 1-k <= t <= 31 + 1 - k
                    lo = max(t0, 1 - k)
                    hi = min(t1, T - k)  # t + k - 1 <= T-1
                    nc.tensor.matmul(
                        PS4[:, (lo - t0) * N : (hi + 1 - t0) * N],
                        WT[:, k * C : (k + 1) * C],
                        H[:, (lo + k - 1) * N : (hi + k) * N],
                        start=False,
                        stop=(k == 2),
                    )
                # ---- f2 (ACT): OUT = PReLU(PS4 + b_tcn, a_out) ----
                OUT = outp.tile([C, TCH * N], F32, tag="out")
                nc.scalar.activation(
                    OUT[:], PS4[:], Prelu, bias=BT[:], alpha=a_out
                )
                nc.sync.dma_start(
                    out_d.ap()[b][:, t0 * N : (t1 + 1) * N], OUT[:]
                )

    nc.compile()
    _BUILD_CACHE[key] = nc
    return nc


def kernel(x, A, w_gcn, b_gcn, w_tcn, b_tcn, a_tcn, a_out):
    from concourse import bass_utils

    x = np.ascontiguousarray(np.asarray(x, dtype=np.float32))
    A = np.asarray(A, dtype=np.float32)
    w_gcn = np.asarray(w_gcn, dtype=np.float32)
    b_gcn = np.asarray(b_gcn, dtype=np.float32)
    w_tcn = np.asarray(w_tcn, dtype=np.float32)
    b_tcn = np.asarray(b_tcn, dtype=np.float32)

    nc = _build(float(np.asarray(a_tcn)), float(np.asarray(a_out)))

    # host-side input prep (marshaling)
    At = A + np.eye(N, dtype=np.float32)  # A + I, used on device
    xe = np.concatenate(
        [x.reshape(B, C, TN), np.ones((B, 1, TN), np.float32)], axis=1
    )  # ones row -> gcn bias via matmul
    WG = np.empty((C + 1, 2 * C), np.float32)
    WG[:C] = w_gcn.transpose(2, 0, 1).reshape(C, 2 * C)  # WG[c, r*64+o]
    WG[C] = b_gcn.reshape(2 * C)
    WT = w_tcn[:, :, :, 0].transpose(1, 2, 0).reshape(C, 3 * C)  # WT[c, k*64+o]
    I64 = np.eye(C, dtype=np.float32)
    I128 = np.eye(N, dtype=np.float32)
    BT = b_tcn.reshape(C, 1)

    in_maps = []
    for c in range(NCORES):
        sl = slice(c * BL, (c + 1) * BL)
        in_maps.append(
            {
                "x": np.ascontiguousarray(xe[sl]),
                "A": np.ascontiguousarray(At[sl]),
                "WG": WG,
                "WT": WT,
                "I64": I64,
                "I128": I128,
                "BT": BT,
            }
        )

    res = bass_utils.run_bass_kernel_spmd(nc, in_maps, core_ids=list(range(NCORES)))
    kernel._last_results = res
    out = np.concatenate([r["out"] for r in res.results], axis=0)
    return (out.reshape(B, C, T, N), A)


# revision 5
# speedup vs baseline: 1.8603x; 1.8603x over previous
"""DMRGCN block kernel for 8 Trainium2 NeuronCores.

Problem: out = PReLU(TCN(PReLU(sum_r (x @ W_r) @ Anorm_r)) + x), where
Anorm = D^-1/2 (A+I) D^-1/2 per (b, r, t) graph of N=128 nodes.

Sharding: pure data parallel over B (64 samples -> 8 cores x 8).

Device pipeline per sample b (layouts: [partition, free]):
  XB  [65, (t n)]   <- DMA x[b] (+ ones row for the gcn bias)
  AB_r [128n, (t m)] <- DMA (A+I)[b, r]   (host adds I)
  DS/D [128n, (r t)]  rowsum -> d = rsqrt(rowsum)  (DVE reduce + Newton)
  mm1  (PE, fp32r): XC[n, (r o)] = x_bt^T @ WG per t  -> PSUM
  e0   (DVE): ZS[n, (r t o)] = XC * d[n]              (PSUM->SBUF)
  mm2  (PE, fp32):  PS2[m, o] = At_r^T @ ZS_rt per (r,t) -> PSUM
  e2   (DVE): Y2 = PS2 * d[m]                          (PSUM->SBUF)
  e3   (POOL): XCT[m, (t o)] = Y2[r=0] + Y2[r=1]
  tr   (PE): PS3[o, m] = XCT_t^T per t -> PSUM
  ph   (ACT): H[o, (t n)] = PReLU(PS3, a_tcn)  fp32r   (PSUM->SBUF)
  tcn  (PE, fp32r): PS4[o, (t n)] = I64 @ x  +  sum_k WT_k @ H(shift k)
  f2   (ACT): OUT = PReLU(PS4 + b_tcn, a_out)          (PSUM->SBUF)
  DMA OUT -> out[b]
"""

import numpy as np

B, R, T, N = 64, 2, 32, 128
C = 64
NCORES = 8
BL = B // NCORES  # samples per core
TN = T * N  # 4096
TCH = 4  # t-chunk size (4 t's = 512 psum cols)
NCH = T // TCH  # 8 chunks

_BUILD_CACHE = {}


def _build(a_tcn: float, a_out: float):
    key = (a_tcn, a_out)
    if key in _BUILD_CACHE:
        return _BUILD_CACHE[key]

    from contextlib import ExitStack

    import concourse.bacc as bacc
    import concourse.mybir as mybir
    import concourse.tile as tile

    dt = mybir.dt
    F32 = dt.float32
    BF16 = dt.bfloat16

    nc = bacc.Bacc("TRN2", target_bir_lowering=False, debug=False)

    x_d = nc.dram_tensor("x", [BL, C + 1, TN], BF16, kind="ExternalInput")
    a_d = nc.dram_tensor("A", [BL, R, T, N, N], BF16, kind="ExternalInput")
    wg_d = nc.dram_tensor("WG", [C + 1, 2 * C], BF16, kind="ExternalInput")
    wt_d = nc.dram_tensor("WT", [C, 3 * C], BF16, kind="ExternalInput")
    i64_d = nc.dram_tensor("I64", [C, C], BF16, kind="ExternalInput")
    i128_d = nc.dram_tensor("I128", [N, N], BF16, kind="ExternalInput")
    bt_d = nc.dram_tensor("BT", [C, 1], F32, kind="ExternalInput")
    out_d = nc.dram_tensor("out", [BL, C, TN], F32, kind="ExternalOutput")

    with tile.TileContext(nc) as tc, ExitStack() as ctx:
        consts = ctx.enter_context(tc.tile_pool(name="consts", bufs=1))
        xp = ctx.enter_context(tc.tile_pool(name="xp", bufs=2))
        ap = ctx.enter_context(tc.tile_pool(name="ap", bufs=3))
        dp = ctx.enter_context(tc.tile_pool(name="dp", bufs=2))
        zsp = ctx.enter_context(tc.tile_pool(name="zsp", bufs=2))
        y2p = ctx.enter_context(tc.tile_pool(name="y2p", bufs=3))
        xctp = ctx.enter_context(tc.tile_pool(name="xctp", bufs=2))
        hp = ctx.enter_context(tc.tile_pool(name="hp", bufs=2))
        outp = ctx.enter_context(tc.tile_pool(name="outp", bufs=3))
        ps1p = ctx.enter_context(tc.tile_pool(name="ps1", bufs=2, space="PSUM"))
        ps2p = ctx.enter_context(tc.tile_pool(name="ps2", bufs=2, space="PSUM"))
        ps3p = ctx.enter_context(tc.tile_pool(name="ps3", bufs=2, space="PSUM"))
        ps4p = ctx.enter_context(tc.tile_pool(name="ps4", bufs=2, space="PSUM"))

        WG = consts.tile([C + 1, 2 * C], BF16)
        nc.sync.dma_start(WG[:], wg_d.ap())
        WT = consts.tile([C, 3 * C], BF16)
        nc.sync.dma_start(WT[:], wt_d.ap())
        I64 = consts.tile([C, C], BF16)
        nc.sync.dma_start(I64[:], i64_d.ap())
        I128 = consts.tile([N, N], BF16)
        nc.sync.dma_start(I128[:], i128_d.ap())
        BT = consts.tile([C, 1], F32)
        nc.sync.dma_start(BT[:], bt_d.ap())

        Prelu = mybir.ActivationFunctionType.Prelu
        Sqrt = mybir.ActivationFunctionType.Sqrt
        mult = mybir.AluOpType.mult
        add = mybir.AluOpType.add

        for b in range(BL):
            XB = xp.tile([C + 1, TN], BF16, tag="xb")
            nc.sync.dma_start(XB[:], x_d.ap()[b])

            ABs = []
            for r in range(R):
                AB = ap.tile([N, TN], BF16, tag="ab")
                nc.sync.dma_start(
                    AB[:].rearrange("n (t m) -> n t m", m=N),
                    a_d.ap()[b, r].rearrange("t n m -> n t m"),
                )
                ABs.append(AB)

            # ---- degree vector d = rsqrt(rowsum(A+I)) : [128, (r t)] ----
            DS = dp.tile([N, R * T], F32, tag="ds")
            for r in range(R):
                nc.vector.reduce_sum(
                    DS[:, r * T : (r + 1) * T],
                    ABs[r][:].rearrange("n (t m) -> n t m", m=N),
                    axis=mybir.AxisListType.X,
                )
            RC = dp.tile([N, R * T], F32, tag="rc")
            nc.vector.reciprocal(RC[:], DS[:])
            D0 = dp.tile([N, R * T], F32, tag="d0")
            nc.scalar.activation(D0[:], RC[:], Sqrt)
            # one Newton step for rsqrt: d = d0*(1.5 - 0.5*s*d0^2)
            T1 = dp.tile([N, R * T], F32, tag="t1")
            nc.vector.tensor_mul(T1[:], D0[:], D0[:])
            nc.vector.tensor_mul(T1[:], T1[:], DS[:])
            nc.vector.tensor_scalar(T1[:], T1[:], -0.5, 1.5, op0=mult, op1=add)
            D = dp.tile([N, R * T], F32, tag="d")
            nc.vector.tensor_mul(D[:], T1[:], D0[:])

            ZS = zsp.tile([N, R * T * C], BF16, tag="zs")
            XCT = xctp.tile([N, T * C], BF16, tag="xct")
            H = hp.tile([C, TN], BF16, tag="h")

            for j in range(NCH):
                t0 = j * TCH
                # ---- mm1: XC[n, (r o)] per t ----
                PS1 = ps1p.tile([N, TCH * 2 * C], F32, tag="ps1")
                for tt in range(TCH):
                    t = t0 + tt
                    nc.tensor.matmul(
                        PS1[:, tt * 128 : (tt + 1) * 128],
                        XB[:, t * N : (t + 1) * N],
                        WG[:],
                        start=True,
                        stop=True,
                    )
                # ---- e0: ZS = XC * d[n] ----
                ps1_v = PS1[:].rearrange("n (t r o) -> n t r o", t=TCH, r=R)
                zs_v = (
                    ZS[:]
                    .rearrange("n (r t o) -> n r t o", r=R, o=C)[
                        :, :, t0 : t0 + TCH
                    ]
                    .rearrange("n r t o -> n t r o")
                )
                d_v = (
                    D[:]
                    .rearrange("n (r t) -> n r t", r=R)[:, :, t0 : t0 + TCH]
                    .rearrange("n r t -> n t r")
                    .unsqueeze(3)
                    .broadcast_to((N, TCH, R, C))
                )
                nc.vector.tensor_tensor(zs_v, ps1_v, d_v, op=mult)

                # ---- mm2 per (r, t): PS2[m, o] ----
                PS2 = ps2p.tile([N, TCH * 2 * C], F32, tag="ps2")
                for tt in range(TCH):
                    t = t0 + tt
                    for r in range(R):
                        nc.tensor.matmul(
                            PS2[:, (tt * 2 + r) * C : (tt * 2 + r + 1) * C],
                            ABs[r][:, t * N : (t + 1) * N],
                            ZS[:, (r * T + t) * C : (r * T + t + 1) * C],
                            start=True,
                            stop=True,
                        )
                # ---- e2: Y2 = PS2 * d[m] ----
                Y2 = y2p.tile([N, TCH * 2 * C], F32, tag="y2")
                ps2_v = PS2[:].rearrange("m (t r o) -> m t r o", t=TCH, r=R)
                y2_v = Y2[:].rearrange("m (t r o) -> m t r o", t=TCH, r=R)
                nc.vector.tensor_tensor(y2_v, ps2_v, d_v, op=mult)
                # ---- e3 (gpsimd): XCT[m, (t o)] = Y2[r0] + Y2[r1] ----
                y2r = Y2[:].rearrange("m (t r o) -> m r t o", t=TCH, r=R)
                nc.gpsimd.tensor_tensor(
                    XCT[:, t0 * C : (t0 + TCH) * C].rearrange(
                        "m (t o) -> m t o", o=C
                    ),
                    y2r[:, 0],
                    y2r[:, 1],
                    op=add,
                )

                # ---- transpose per t: PS3[o, m] ----
                PS3 = ps3p.tile([C, TCH * N], BF16, tag="ps3")
                for tt in range(TCH):
                    t = t0 + tt
                    nc.tensor.transpose(
                        PS3[:, tt * N : (tt + 1) * N],
                        XCT[:, t * C : (t + 1) * C],
                        I128[:],
                    )
                # ---- prelu-h (ACT): H = PReLU(PS3, a_tcn) ----
                nc.scalar.activation(
                    H[:, t0 * N : (t0 + TCH) * N], PS3[:], Prelu, alpha=a_tcn
                )

            # ---- tcn + residual per chunk ----
            for j in range(NCH):
                t0 = j * TCH
                t1 = t0 + TCH - 1  # inclusive
                PS4 = ps4p.tile([C, TCH * N], F32, tag="ps4")
                nc.tensor.matmul(
                    PS4[:],
                    I64[:],
                    XB[:C, t0 * N : (t1 + 1) * N],
                    start=True,
                    stop=False,
                )
                for k in range(3):
                    # valid out t range for tap k: 1-k <= t <= 31 + 1 - k
                    lo = max(t0, 1 - k)
                    hi = min(t1, T - k)  # t + k - 1 <= T-1
                    nc.tensor.matmul(
                        PS4[:, (lo - t0) * N : (hi + 1 - t0) * N],
                        WT[:, k * C : (k + 1) * C],
                        H[:, (lo + k - 1) * N : (hi + k) * N],
                        start=False,
                        stop=(k == 2),
                    )
                # ---- f2 (ACT): OUT = PReLU(PS4 + b_tcn, a_out) ----
                OUT = outp.tile([C, TCH * N], F32, tag="out")
                nc.scalar.activation(
                    OUT[:], PS4[:], Prelu, bias=BT[:], alpha=a_out
                )
                nc.sync.dma_start(
                    out_d.ap()[b][:, t0 * N : (t1 + 1) * N], OUT[:]
                )

    nc.compile()
    _BUILD_CACHE[key] = nc
    return nc


def kernel(x, A, w_gcn, b_gcn, w_tcn, b_tcn, a_tcn, a_out):
    from concourse import bass_utils

    x = np.ascontiguousarray(np.asarray(x, dtype=np.float32))
    A = np.asarray(A, dtype=np.float32)
    w_gcn = np.asarray(w_gcn, dtype=np.float32)
    b_gcn = np.asarray(b_gcn, dtype=np.float32)
    w_tcn = np.asarray(w_tcn, dtype=np.float32)
    b_tcn = np.asarray(b_tcn, dtype=np.float32)

    nc = _build(float(np.asarray(a_tcn)), float(np.asarray(a_out)))

    import ml_dtypes

    bf16 = ml_dtypes.bfloat16

    # host-side input prep (marshaling)
    At = (A + np.eye(N, dtype=np.float32)).astype(bf16)  # A + I
    xe = np.concatenate(
        [x.reshape(B, C, TN), np.ones((B, 1, TN), np.float32)], axis=1
    ).astype(bf16)  # ones row -> gcn bias via matmul
    WG = np.empty((C + 1, 2 * C), np.float32)
    WG[:C] = w_gcn.transpose(2, 0, 1).reshape(C, 2 * C)  # WG[c, r*64+o]
    WG[C] = b_gcn.reshape(2 * C)
    WG = WG.astype(bf16)
    WT = (
        w_tcn[:, :, :, 0].transpose(1, 2, 0).reshape(C, 3 * C).astype(bf16)
    )  # WT[c, k*64+o]
    I64 = np.eye(C, dtype=np.float32).astype(bf16)
    I128 = np.eye(N, dtype=np.float32).astype(bf16)
    BT = b_tcn.reshape(C, 1)

    in_maps = []
    for c in range(NCORES):
        sl = slice(c * BL, (c + 1) * BL)
        in_maps.append(
            {
                "x": np.ascontiguousarray(xe[sl]),
                "A": np.ascontiguousarray(At[sl]),
                "WG": WG,
                "WT": WT,
                "I64": I64,
                "I128": I128,
                "BT": BT,
            }
        )

    res = bass_utils.run_bass_kernel_spmd(nc, in_maps, core_ids=list(range(NCORES)))
    kernel._last_results = res
    out = np.concatenate([r["out"] for r in res.results], axis=0)
    return (out.reshape(B, C, T, N), A)


# revision 7
# speedup vs baseline: 2.0849x; 1.1208x over previous
"""DMRGCN block kernel for 8 Trainium2 NeuronCores.

Problem: out = PReLU(TCN(PReLU(sum_r (x @ W_r) @ Anorm_r)) + x), where
Anorm = D^-1/2 (A+I) D^-1/2 per (b, r, t) graph of N=128 nodes.

Sharding: pure data parallel over B (64 samples -> 8 cores x 8).

Device pipeline per sample b (layouts: [partition, free], bf16 matmuls):
  XB  [65, (t n)]    <- DMA x[b] (+ ones row for the gcn bias)
  AB  [128n, (r t m)] <- DMA (A+I)[b] (host adds I + pre-transposes)
  D   [128n, (r t)]   d = rsqrt(rowsum(A+I))  (DVE reduce+recip, ACT sqrt)
  mm1 (PE): XC[n, (r o)] = x_bt^T @ WG per t -> PSUM
  e0  (DVE): ZS[n, (r t o)] = XC * d[n]          (PSUM->SBUF)
  mm2 (PE): PS2[m, o] = At_rt^T @ ZS_rt per (r,t) -> PSUM
  e2  (DVE): Y2 = PS2 * d[m]                     (PSUM->SBUF)
  e3  (POOL): XCT[m, (t o)] = Y2[r=0] + Y2[r=1]
  tr  (PE): PS3[o, m] = XCT_t^T per t -> PSUM
  ph  (ACT): H2[0:64] = PReLU(PS3, a_tcn)        (PSUM->SBUF)
  tcn (PE): PS4[o, (t n)] = [W_k1; I] @ [h; x] + W_k0 @ h(-1) + W_k2 @ h(+1)
  f2  (ACT): OUT = PReLU(PS4 + b_tcn, a_out)     (PSUM->SBUF)
  DMA OUT -> out[b]
"""

import numpy as np

B, R, T, N = 64, 2, 32, 128
C = 64
NCORES = 8
BL = B // NCORES  # samples per core
TN = T * N  # 4096
TCH = 4  # t-chunk size (4 t's = 512 psum cols)
NCH = T // TCH  # 8 chunks

_BUILD_CACHE = {}


def _build(a_tcn: float, a_out: float):
    key = (a_tcn, a_out)
    if key in _BUILD_CACHE:
        return _BUILD_CACHE[key]

    from contextlib import ExitStack

    import concourse.bacc as bacc
    import concourse.mybir as mybir
    import concourse.tile as tile

    dt = mybir.dt
    F32 = dt.float32
    BF16 = dt.bfloat16

    nc = bacc.Bacc("TRN2", target_bir_lowering=False, debug=False)

    x_d = nc.dram_tensor("x", [BL, C + 1, TN], BF16, kind="ExternalInput")
    a_d = nc.dram_tensor("A", [BL, N, R * T * N], BF16, kind="ExternalInput")
    wg_d = nc.dram_tensor("WG", [C + 1, 2 * C], BF16, kind="ExternalInput")
    wt_d = nc.dram_tensor("WT", [C, 3 * C], BF16, kind="ExternalInput")
    wta_d = nc.dram_tensor("WTA", [2 * C, C], BF16, kind="ExternalInput")
    i128_d = nc.dram_tensor("I128", [N, N], BF16, kind="ExternalInput")
    bt_d = nc.dram_tensor("BT", [C, 1], F32, kind="ExternalInput")
    out_d = nc.dram_tensor("out", [BL, C, TN], F32, kind="ExternalOutput")

    with tile.TileContext(nc) as tc, ExitStack() as ctx:
        consts = ctx.enter_context(tc.tile_pool(name="consts", bufs=1))
        xp = ctx.enter_context(tc.tile_pool(name="xp", bufs=3))
        app = ctx.enter_context(tc.tile_pool(name="app", bufs=3))
        dp = ctx.enter_context(tc.tile_pool(name="dp", bufs=3))
        zsp = ctx.enter_context(tc.tile_pool(name="zsp", bufs=2))
        y2p = ctx.enter_context(tc.tile_pool(name="y2p", bufs=3))
        xctp = ctx.enter_context(tc.tile_pool(name="xctp", bufs=2))
        hp = ctx.enter_context(tc.tile_pool(name="hp", bufs=3))
        outp = ctx.enter_context(tc.tile_pool(name="outp", bufs=3))
        ps1p = ctx.enter_context(tc.tile_pool(name="ps1", bufs=2, space="PSUM"))
        ps2p = ctx.enter_context(tc.tile_pool(name="ps2", bufs=2, space="PSUM"))
        ps3p = ctx.enter_context(tc.tile_pool(name="ps3", bufs=2, space="PSUM"))
        ps4p = ctx.enter_context(tc.tile_pool(name="ps4", bufs=2, space="PSUM"))

        WG = consts.tile([C + 1, 2 * C], BF16)
        nc.sync.dma_start(WG[:], wg_d.ap())
        WT = consts.tile([C, 3 * C], BF16)
        nc.sync.dma_start(WT[:], wt_d.ap())
        WTA = consts.tile([2 * C, C], BF16)
        nc.sync.dma_start(WTA[:], wta_d.ap())
        I128 = consts.tile([N, N], BF16)
        nc.sync.dma_start(I128[:], i128_d.ap())
        BT = consts.tile([C, 1], F32)
        nc.sync.dma_start(BT[:], bt_d.ap())

        Prelu = mybir.ActivationFunctionType.Prelu
        Sqrt = mybir.ActivationFunctionType.Sqrt
        mult = mybir.AluOpType.mult
        add = mybir.AluOpType.add

        state = {}

        def phase_a(b):
            """DMAs for sample b + the d-chain (prefetched ahead)."""
            XB = xp.tile([C + 1, TN], BF16, tag="xb")
            nc.sync.dma_start(XB[:], x_d.ap()[b])
            AB = app.tile([N, R * T * N], BF16, tag="ab")
            nc.sync.dma_start(AB[:], a_d.ap()[b])
            H2 = hp.tile([2 * C, TN], BF16, tag="h")
            nc.sync.dma_start(H2[C:, :], x_d.ap()[b][:C])  # residual x
            DS = dp.tile([N, R * T], F32, tag="ds")
            nc.vector.reduce_sum(
                DS[:],
                AB[:].rearrange("n (rt m) -> n rt m", m=N),
                axis=mybir.AxisListType.X,
            )
            RC = dp.tile([N, R * T], F32, tag="rc")
            nc.vector.reciprocal(RC[:], DS[:])
            D = dp.tile([N, R * T], F32, tag="d")
            nc.scalar.activation(D[:], RC[:], Sqrt)
            state[b] = (XB, AB, H2, D)

        def phase_b(b):
            XB, AB, H2, D = state.pop(b)
            ZS = zsp.tile([N, R * T * C], BF16, tag="zs")
            XCT = xctp.tile([N, T * C], BF16, tag="xct")

            for j in range(NCH):
                t0 = j * TCH
                # ---- mm1: XC[n, (r o)] per t ----
                PS1 = ps1p.tile([N, TCH * 2 * C], F32, tag="ps1")
                for tt in range(TCH):
                    t = t0 + tt
                    nc.tensor.matmul(
                        PS1[:, tt * 128 : (tt + 1) * 128],
                        XB[:, t * N : (t + 1) * N],
                        WG[:],
                        start=True,
                        stop=True,
                    )
                # ---- e0: ZS = XC * d[n] ----
                ps1_v = PS1[:].rearrange("n (t r o) -> n t r o", t=TCH, r=R)
                zs_v = (
                    ZS[:]
                    .rearrange("n (r t o) -> n r t o", r=R, o=C)[
                        :, :, t0 : t0 + TCH
                    ]
                    .rearrange("n r t o -> n t r o")
                )
                d_v = (
                    D[:]
                    .rearrange("n (r t) -> n r t", r=R)[:, :, t0 : t0 + TCH]
                    .rearrange("n r t -> n t r")
                    .unsqueeze(3)
                    .broadcast_to((N, TCH, R, C))
                )
                nc.vector.tensor_tensor(zs_v, ps1_v, d_v, op=mult)

                # ---- mm2 per (r, t): PS2[m, o] ----
                PS2 = ps2p.tile([N, TCH * 2 * C], F32, tag="ps2")
                for tt in range(TCH):
                    t = t0 + tt
                    for r in range(R):
                        nc.tensor.matmul(
                            PS2[:, (tt * 2 + r) * C : (tt * 2 + r + 1) * C],
                            AB[:, (r * T + t) * N : (r * T + t + 1) * N],
                            ZS[:, (r * T + t) * C : (r * T + t + 1) * C],
                            start=True,
                            stop=True,
                        )
                # ---- e2: Y2 = PS2 * d[m] ----
                Y2 = y2p.tile([N, TCH * 2 * C], F32, tag="y2")
                ps2_v = PS2[:].rearrange("m (t r o) -> m t r o", t=TCH, r=R)
                y2_v = Y2[:].rearrange("m (t r o) -> m t r o", t=TCH, r=R)
                nc.vector.tensor_tensor(y2_v, ps2_v, d_v, op=mult)
                # ---- e3 (gpsimd): XCT[m, (t o)] = Y2[r0] + Y2[r1] ----
                y2r = Y2[:].rearrange("m (t r o) -> m r t o", t=TCH, r=R)
                nc.gpsimd.tensor_tensor(
                    XCT[:, t0 * C : (t0 + TCH) * C].rearrange(
                        "m (t o) -> m t o", o=C
                    ),
                    y2r[:, 0],
                    y2r[:, 1],
                    op=add,
                )

                # ---- transpose per t: PS3[o, m] ----
                PS3 = ps3p.tile([C, TCH * N], BF16, tag="ps3")
                for tt in range(TCH):
                    t = t0 + tt
                    nc.tensor.transpose(
                        PS3[:, tt * N : (tt + 1) * N],
                        XCT[:, t * C : (t + 1) * C],
                        I128[:],
                    )
                # ---- prelu-h (ACT): H2 top = PReLU(PS3, a_tcn) ----
                nc.scalar.activation(
                    H2[:C, t0 * N : (t0 + TCH) * N], PS3[:], Prelu, alpha=a_tcn
                )

            # ---- tcn (+ residual inside the k=1 tap) per chunk ----
            for j in range(NCH):
                t0 = j * TCH
                t1 = t0 + TCH - 1  # inclusive
                PS4 = ps4p.tile([C, TCH * N], F32, tag="ps4")
                nc.tensor.matmul(
                    PS4[:],
                    WTA[:],
                    H2[:, t0 * N : (t1 + 1) * N],
                    start=True,
                    stop=False,
                )
                for k in (0, 2):
                    lo = max(t0, 1 - k)
                    hi = min(t1, T - k)  # t + k - 1 <= T-1
                    nc.tensor.matmul(
                        PS4[:, (lo - t0) * N : (hi + 1 - t0) * N],
                        WT[:, k * C : (k + 1) * C],
                        H2[:C, (lo + k - 1) * N : (hi + k) * N],
                        start=False,
                        stop=(k == 2),
                    )
                # ---- f2 (ACT): OUT = PReLU(PS4 + b_tcn, a_out) ----
                OUT = outp.tile([C, TCH * N], F32, tag="out")
                nc.scalar.activation(
                    OUT[:], PS4[:], Prelu, bias=BT[:], alpha=a_out
                )
                nc.sync.dma_start(
                    out_d.ap()[b][:, t0 * N : (t1 + 1) * N], OUT[:]
                )

        phase_a(0)
        if BL > 1:
            phase_a(1)
        for b in range(BL):
            if b + 2 < BL:
                phase_a(b + 2)
            phase_b(b)

    nc.compile()
    _BUILD_CACHE[key] = nc
    return nc


def kernel(x, A, w_gcn, b_gcn, w_tcn, b_tcn, a_tcn, a_out):
    import ml_dtypes
    from concourse import bass_utils

    bf16 = ml_dtypes.bfloat16

    x = np.ascontiguousarray(np.asarray(x, dtype=np.float32))
    A = np.asarray(A, dtype=np.float32)
    w_gcn = np.asarray(w_gcn, dtype=np.float32)
    b_gcn = np.asarray(b_gcn, dtype=np.float32)
    w_tcn = np.asarray(w_tcn, dtype=np.float32)
    b_tcn = np.asarray(b_tcn, dtype=np.float32)

    nc = _build(float(np.asarray(a_tcn)), float(np.asarray(a_out)))

    # host-side input prep (marshaling)
    At = A + np.eye(N, dtype=np.float32)  # A + I
    # [B, r, t, n, m] -> [B, n, (r t m)] so the device DMA is contiguous
    Atp = (
        np.ascontiguousarray(At.transpose(0, 3, 1, 2, 4))
        .reshape(B, N, R * T * N)
        .astype(bf16)
    )
    xe = np.concatenate(
        [x.reshape(B, C, TN), np.ones((B, 1, TN), np.float32)], axis=1
    ).astype(bf16)  # ones row -> gcn bias via matmul
    WG = np.empty((C + 1, 2 * C), np.float32)
    WG[:C] = w_gcn.transpose(2, 0, 1).reshape(C, 2 * C)  # WG[c, r*64+o]
    WG[C] = b_gcn.reshape(2 * C)
    WG = WG.astype(bf16)
    WT = (
        w_tcn[:, :, :, 0].transpose(1, 2, 0).reshape(C, 3 * C).astype(bf16)
    )  # WT[c, k*64+o]
    WTA = np.concatenate(
        [w_tcn[:, :, 1, 0].T, np.eye(C, dtype=np.float32)], axis=0
    ).astype(bf16)  # [W_k1; I64] for the fused k=1 + residual matmul
    I128 = np.eye(N, dtype=np.float32).astype(bf16)
    BT = b_tcn.reshape(C, 1)

    in_maps = []
    for c in range(NCORES):
        sl = slice(c * BL, (c + 1) * BL)
        in_maps.append(
            {
                "x": np.ascontiguousarray(xe[sl]),
                "A": np.ascontiguousarray(Atp[sl]),
                "WG": WG,
                "WT": WT,
                "WTA": WTA,
                "I128": I128,
                "BT": BT,
            }
        )

    res = bass_utils.run_bass_kernel_spmd(nc, in_maps, core_ids=list(range(NCORES)))
    kernel._last_results = res
    out = np.concatenate([r["out"] for r in res.results], axis=0)
    return (out.reshape(B, C, T, N), A)


# revision 8
# speedup vs baseline: 2.1733x; 1.0424x over previous
"""DMRGCN block kernel for 8 Trainium2 NeuronCores.

Problem: out = PReLU(TCN(PReLU(sum_r (x @ W_r) @ Anorm_r)) + x), where
Anorm = D^-1/2 (A+I) D^-1/2 per (b, r, t) graph of N=128 nodes.

Sharding: pure data parallel over B (64 samples -> 8 cores x 8).

Device pipeline per sample b (layouts: [partition, free], bf16 matmuls):
  XB  [65, (t n)]    <- DMA x[b] (+ ones row for the gcn bias)
  AB  [128n, (r t m)] <- DMA (A+I)[b] (host adds I + pre-transposes)
  D   [128n, (r t)]   d = rsqrt(rowsum(A+I))  (DVE reduce+recip, ACT sqrt)
  mm1 (PE): XC[n, (r o)] = x_bt^T @ WG per t -> PSUM
  e0  (DVE): ZS[n, (r t o)] = XC * d[n]          (PSUM->SBUF)
  mm2 (PE): PS2[m, o] = At_rt^T @ ZS_rt per (r,t) -> PSUM
  e2  (DVE): Y2 = PS2 * d[m]                     (PSUM->SBUF)
  e3  (POOL): XCT[m, (t o)] = Y2[r=0] + Y2[r=1]
  tr  (PE): PS3[o, m] = XCT_t^T per t -> PSUM
  ph  (ACT): H2[0:64] = PReLU(PS3, a_tcn)        (PSUM->SBUF)
  tcn (PE): PS4[o, (t n)] = [W_k1; I] @ [h; x] + W_k0 @ h(-1) + W_k2 @ h(+1)
  f2  (ACT): OUT = PReLU(PS4 + b_tcn, a_out)     (PSUM->SBUF)
  DMA OUT -> out[b]
"""

import numpy as np

B, R, T, N = 64, 2, 32, 128
C = 64
NCORES = 8
BL = B // NCORES  # samples per core
TN = T * N  # 4096
TCH = 4  # t-chunk size (4 t's = 512 psum cols)
NCH = T // TCH  # 8 chunks

_BUILD_CACHE = {}


def _build(a_tcn: float, a_out: float):
    key = (a_tcn, a_out)
    if key in _BUILD_CACHE:
        return _BUILD_CACHE[key]

    from contextlib import ExitStack

    import concourse.bacc as bacc
    import concourse.mybir as mybir
    import concourse.tile as tile

    dt = mybir.dt
    F32 = dt.float32
    BF16 = dt.bfloat16

    nc = bacc.Bacc("TRN2", target_bir_lowering=False, debug=False)

    x_d = nc.dram_tensor("x", [BL, C + 1, TN], BF16, kind="ExternalInput")
    a_d = nc.dram_tensor("A", [BL, N, R * T * N], BF16, kind="ExternalInput")
    wg_d = nc.dram_tensor("WG", [C + 1, 2 * C], BF16, kind="ExternalInput")
    wt_d = nc.dram_tensor("WT", [C, 3 * C], BF16, kind="ExternalInput")
    wta_d = nc.dram_tensor("WTA", [2 * C, C], BF16, kind="ExternalInput")
    i128_d = nc.dram_tensor("I128", [N, N], BF16, kind="ExternalInput")
    bt_d = nc.dram_tensor("BT", [C, 1], F32, kind="ExternalInput")
    out_d = nc.dram_tensor("out", [BL, C, TN], F32, kind="ExternalOutput")

    with tile.TileContext(nc) as tc, ExitStack() as ctx:
        consts = ctx.enter_context(tc.tile_pool(name="consts", bufs=1))
        xp = ctx.enter_context(tc.tile_pool(name="xp", bufs=3))
        app = ctx.enter_context(tc.tile_pool(name="app", bufs=3))
        dp = ctx.enter_context(tc.tile_pool(name="dp", bufs=3))
        zsp = ctx.enter_context(tc.tile_pool(name="zsp", bufs=2))
        y2p = ctx.enter_context(tc.tile_pool(name="y2p", bufs=3))
        xctp = ctx.enter_context(tc.tile_pool(name="xctp", bufs=2))
        hp = ctx.enter_context(tc.tile_pool(name="hp", bufs=3))
        outp = ctx.enter_context(tc.tile_pool(name="outp", bufs=3))
        ps1p = ctx.enter_context(tc.tile_pool(name="ps1", bufs=2, space="PSUM"))
        ps2p = ctx.enter_context(tc.tile_pool(name="ps2", bufs=2, space="PSUM"))
        ps3p = ctx.enter_context(tc.tile_pool(name="ps3", bufs=2, space="PSUM"))
        ps4p = ctx.enter_context(tc.tile_pool(name="ps4", bufs=2, space="PSUM"))

        WG = consts.tile([C + 1, 2 * C], BF16)
        nc.sync.dma_start(WG[:], wg_d.ap())
        WT = consts.tile([C, 3 * C], BF16)
        nc.sync.dma_start(WT[:], wt_d.ap())
        WTA = consts.tile([2 * C, C], BF16)
        nc.sync.dma_start(WTA[:], wta_d.ap())
        I128 = consts.tile([N, N], BF16)
        nc.sync.dma_start(I128[:], i128_d.ap())
        BT = consts.tile([C, 1], F32)
        nc.sync.dma_start(BT[:], bt_d.ap())

        Prelu = mybir.ActivationFunctionType.Prelu
        Sqrt = mybir.ActivationFunctionType.Sqrt
        mult = mybir.AluOpType.mult
        add = mybir.AluOpType.add

        state = {}

        def phase_a(b):
            """DMAs for sample b + the d-chain (prefetched ahead)."""
            XB = xp.tile([C + 1, TN], BF16, tag="xb")
            nc.sync.dma_start(XB[:], x_d.ap()[b])
            AB = app.tile([N, R * T * N], BF16, tag="ab")
            nc.sync.dma_start(AB[:], a_d.ap()[b])
            H2 = hp.tile([2 * C, TN], BF16, tag="h")
            nc.sync.dma_start(H2[C:, :], x_d.ap()[b][:C])  # residual x
            DS = dp.tile([N, R * T], F32, tag="ds")
            nc.vector.reduce_sum(
                DS[:],
                AB[:].rearrange("n (rt m) -> n rt m", m=N),
                axis=mybir.AxisListType.X,
            )
            RC = dp.tile([N, R * T], F32, tag="rc")
            nc.vector.reciprocal(RC[:], DS[:])
            D = dp.tile([N, R * T], F32, tag="d")
            nc.scalar.activation(D[:], RC[:], Sqrt)
            state[b] = (XB, AB, H2, D)

        def phase_b(b):
            XB, AB, H2, D = state.pop(b)
            ZS = zsp.tile([N, R * T * C], BF16, tag="zs")
            XCT = xctp.tile([N, T * C], BF16, tag="xct")

            def d_bcast(t0):
                return (
                    D[:]
                    .rearrange("n (r t) -> n r t", r=R)[:, :, t0 : t0 + TCH]
                    .rearrange("n r t -> n t r")
                    .unsqueeze(3)
                    .broadcast_to((N, TCH, R, C))
                )

            def emit_mm1_e0(j):
                t0 = j * TCH
                PS1 = ps1p.tile([N, TCH * 2 * C], F32, tag="ps1")
                for tt in range(TCH):
                    t = t0 + tt
                    nc.tensor.matmul(
                        PS1[:, tt * 128 : (tt + 1) * 128],
                        XB[:, t * N : (t + 1) * N],
                        WG[:],
                        start=True,
                        stop=True,
                    )
                ps1_v = PS1[:].rearrange("n (t r o) -> n t r o", t=TCH, r=R)
                zs_v = (
                    ZS[:]
                    .rearrange("n (r t o) -> n r t o", r=R, o=C)[
                        :, :, t0 : t0 + TCH
                    ]
                    .rearrange("n r t o -> n t r o")
                )
                nc.vector.tensor_tensor(zs_v, ps1_v, d_bcast(t0), op=mult)

            def emit_mm2_e2_e3(j):
                t0 = j * TCH
                PS2 = ps2p.tile([N, TCH * 2 * C], F32, tag="ps2")
                for tt in range(TCH):
                    t = t0 + tt
                    for r in range(R):
                        nc.tensor.matmul(
                            PS2[:, (tt * 2 + r) * C : (tt * 2 + r + 1) * C],
                            AB[:, (r * T + t) * N : (r * T + t + 1) * N],
                            ZS[:, (r * T + t) * C : (r * T + t + 1) * C],
                            start=True,
                            stop=True,
                        )
                Y2 = y2p.tile([N, TCH * 2 * C], F32, tag="y2")
                ps2_v = PS2[:].rearrange("m (t r o) -> m t r o", t=TCH, r=R)
                y2_v = Y2[:].rearrange("m (t r o) -> m t r o", t=TCH, r=R)
                nc.vector.tensor_tensor(y2_v, ps2_v, d_bcast(t0), op=mult)
                y2r = Y2[:].rearrange("m (t r o) -> m r t o", t=TCH, r=R)
                nc.gpsimd.tensor_tensor(
                    XCT[:, t0 * C : (t0 + TCH) * C].rearrange(
                        "m (t o) -> m t o", o=C
                    ),
                    y2r[:, 0],
                    y2r[:, 1],
                    op=add,
                )

            def emit_transp_prelu(j):
                t0 = j * TCH
                PS3 = ps3p.tile([C, TCH * N], BF16, tag="ps3")
                for tt in range(TCH):
                    t = t0 + tt
                    nc.tensor.transpose(
                        PS3[:, tt * N : (tt + 1) * N],
                        XCT[:, t * C : (t + 1) * C],
                        I128[:],
                    )
                nc.scalar.activation(
                    H2[:C, t0 * N : (t0 + TCH) * N], PS3[:], Prelu, alpha=a_tcn
                )

            def emit_tcn(j):
                t0 = j * TCH
                t1 = t0 + TCH - 1  # inclusive
                PS4 = ps4p.tile([C, TCH * N], F32, tag="ps4")
                nc.tensor.matmul(
                    PS4[:],
                    WTA[:],
                    H2[:, t0 * N : (t1 + 1) * N],
                    start=True,
                    stop=False,
                )
                for k in (0, 2):
                    lo = max(t0, 1 - k)
                    hi = min(t1, T - k)  # t + k - 1 <= T-1
                    nc.tensor.matmul(
                        PS4[:, (lo - t0) * N : (hi + 1 - t0) * N],
                        WT[:, k * C : (k + 1) * C],
                        H2[:C, (lo + k - 1) * N : (hi + k) * N],
                        start=False,
                        stop=(k == 2),
                    )
                OUT = outp.tile([C, TCH * N], F32, tag="out")
                nc.scalar.activation(
                    OUT[:], PS4[:], Prelu, bias=BT[:], alpha=a_out
                )
                nc.sync.dma_start(
                    out_d.ap()[b][:, t0 * N : (t1 + 1) * N], OUT[:]
                )

            # software pipeline: PE always has ready work
            for j in range(NCH + 4):
                if j < NCH:
                    emit_mm1_e0(j)
                if 1 <= j < NCH + 1:
                    emit_mm2_e2_e3(j - 1)
                if 3 <= j < NCH + 3:
                    emit_transp_prelu(j - 3)
                if 4 <= j:
                    emit_tcn(j - 4)

        phase_a(0)
        if BL > 1:
            phase_a(1)
        for b in range(BL):
            if b + 2 < BL:
                phase_a(b + 2)
            phase_b(b)

    nc.compile()
    _BUILD_CACHE[key] = nc
    return nc


def kernel(x, A, w_gcn, b_gcn, w_tcn, b_tcn, a_tcn, a_out):
    import ml_dtypes
    from concourse import bass_utils

    bf16 = ml_dtypes.bfloat16

    x = np.ascontiguousarray(np.asarray(x, dtype=np.float32))
    A = np.asarray(A, dtype=np.float32)
    w_gcn = np.asarray(w_gcn, dtype=np.float32)
    b_gcn = np.asarray(b_gcn, dtype=np.float32)
    w_tcn = np.asarray(w_tcn, dtype=np.float32)
    b_tcn = np.asarray(b_tcn, dtype=np.float32)

    nc = _build(float(np.asarray(a_tcn)), float(np.asarray(a_out)))

    # host-side input prep (marshaling)
    At = A + np.eye(N, dtype=np.float32)  # A + I
    # [B, r, t, n, m] -> [B, n, (r t m)] so the device DMA is contiguous
    Atp = (
        np.ascontiguousarray(At.transpose(0, 3, 1, 2, 4))
        .reshape(B, N, R * T * N)
        .astype(bf16)
    )
    xe = np.concatenate(
        [x.reshape(B, C, TN), np.ones((B, 1, TN), np.float32)], axis=1
    ).astype(bf16)  # ones row -> gcn bias via matmul
    WG = np.empty((C + 1, 2 * C), np.float32)
    WG[:C] = w_gcn.transpose(2, 0, 1).reshape(C, 2 * C)  # WG[c, r*64+o]
    WG[C] = b_gcn.reshape(2 * C)
    WG = WG.astype(bf16)
    WT = (
        w_tcn[:, :, :, 0].transpose(1, 2, 0).reshape(C, 3 * C).astype(bf16)
    )  # WT[c, k*64+o]
    WTA = np.concatenate(
        [w_tcn[:, :, 1, 0].T, np.eye(C, dtype=np.float32)], axis=0
    ).astype(bf16)  # [W_k1; I64] for the fused k=1 + residual matmul
    I128 = np.eye(N, dtype=np.float32).astype(bf16)
    BT = b_tcn.reshape(C, 1)

    in_maps = []
    for c in range(NCORES):
        sl = slice(c * BL, (c + 1) * BL)
        in_maps.append(
            {
                "x": np.ascontiguousarray(xe[sl]),
                "A": np.ascontiguousarray(Atp[sl]),
                "WG": WG,
                "WT": WT,
                "WTA": WTA,
                "I128": I128,
                "BT": BT,
            }
        )

    res = bass_utils.run_bass_kernel_spmd(nc, in_maps, core_ids=list(range(NCORES)))
    kernel._last_results = res
    out = np.concatenate([r["out"] for r in res.results], axis=0)
    return (out.reshape(B, C, T, N), A)
